# revision 24
# baseline (speedup 1.0000x reference)
"""Trainium2 Bass kernel for nn_DirectionalDiagram.

out[f, i, j] = X[f, i] + Y[f, j] + x[i, j]        f in [64], i,j in [1024]
  X[f, i] = (cos(t_f) - idx[i]) * 0.5 * cos(t_f) = 0.5c^2 - 0.5c*idx[i]
  Y[f, j] = (sin(t_f) - idx[j]) * 0.5 * sin(t_f) = 0.5s^2 - 0.5s*idx[j]
  idx[i]  = (i - 511.5) / (1024 * sqrt(2))

Sharding: the filter axis is split across the 8 NeuronCores (8 filters per
core); x is replicated.  Per core the kernel is output-bandwidth bound
(32 MiB of f32 writes); the whole computation is a single fused DVE
scalar_tensor_tensor per [128, 1024] output tile:
    out_tile = (x_tile + X_col[f,b]) + YB[f]
where X_col is a per-partition scalar column and YB[f] is Y[f, :]
broadcast across partitions.  X_col and YB are produced by TensorE
matmuls from host-prepared affine coefficients (idx columns x per-filter
(-0.5s, 0.5s^2) etc.) into PSUM + ScalarE copies to SBUF, so the DVE
(and its SBUF ports) runs nothing but the 64 fused main-loop ops.
Output DMAs are byte-balanced over three DMA issue paths (sync HWDGE,
scalar HWDGE, gpsimd SWDGE) that share the 16 SDMA engines.
"""

import numpy as np

W = 1024          # image side
P = 128           # SBUF partitions
NB = W // P       # 8 row-blocks
F_TOTAL = 64
N_CORES = 8
F_LOC = F_TOTAL // N_CORES   # 8 filters per core
GH = 4            # row-blocks per output DMA (2 MiB per dma_start)
HN = 512          # matmul free-dim chunk (one PSUM bank)
# coef input [9, 2240]: per-filter Y coeffs | Y basis | X basis | X coeffs
C_LY = 0          # rows 0-1, cols [0:1024):     lhsT_Y[., f*128+p]
C_RY = 1024       # rows 0-1, cols [1024:2048):  rhs_Y = [idx; ones]
C_LX = 2048       # rows 0-8, cols [2048:2176):  lhsT_X = [ones; idx cols]
C_RX = 2176       # rows 0-8, cols [2176:2240):  rhs_X coeffs
COEFW = C_RX + F_LOC * NB

TRACE = False     # set by test harness to capture an NTFF profile
LAST_RESULT = None

_module_cache = {}


def _build_module():
    import concourse.bacc as bacc
    import concourse.mybir as mybir
    from concourse import tile

    fp32 = mybir.dt.float32
    AOP = mybir.AluOpType

    nc = bacc.Bacc("TRN2", target_bir_lowering=False, debug=False)
    x_d = nc.dram_tensor("x", [P, NB * W], fp32, kind="ExternalInput").ap()
    coef_d = nc.dram_tensor("coef", [9, COEFW], fp32, kind="ExternalInput").ap()
    out_d = nc.dram_tensor("out", [F_LOC, W, W], fp32, kind="ExternalOutput").ap()

    with tile.TileContext(nc) as tc:
        with (
            tc.tile_pool(name="const", bufs=1) as cpool,
            tc.tile_pool(name="outp", bufs=6) as opool,
            tc.tile_pool(name="pxc", bufs=1, space="PSUM") as pxc,
            tc.tile_pool(name="pyb", bufs=4, space="PSUM") as pyb,
        ):
            # ---- coef first on the sync ring: it gates everything ----
            coef_sb = cpool.tile([9, COEFW], fp32)
            nc.sync.dma_start(out=coef_sb[:, :], in_=coef_d[:, :])

            # ---- x (host-pretransposed to [128, 8*1024]) in quarters on
            # the scalar ring ----
            x_sb = cpool.tile([P, NB * W], fp32)
            QW = NB * W // 4
            for q in range(4):
                nc.scalar.dma_start(
                    out=x_sb[:, q * QW : (q + 1) * QW],
                    in_=x_d[:, q * QW : (q + 1) * QW],
                )

            # ---- X columns xc[p, f*NB+b] via one K=9 matmul:
            # [ones; idxcol_b'] rows x host coeffs (0.5c^2, -0.5c) ----
            psx = pxc.tile([P, F_LOC * NB], fp32)
            nc.tensor.matmul(
                psx[:, :],
                coef_sb[:, C_LX : C_LX + P],
                coef_sb[:, C_RX : C_RX + F_LOC * NB],
                start=True, stop=True,
            )
            xc = cpool.tile([P, F_LOC * NB], fp32)
            nc.scalar.copy(xc[:, :], psx[:, :])

            # ---- YB[f] = Y[f, :] broadcast to 128 partitions via K=2
            # matmuls: [-0.5s_f; 0.5s_f^2] rows x [idx; ones] ----
            yb = cpool.tile([P, F_LOC * W], fp32)
            for f in range(F_LOC):
                for hf in range(W // HN):
                    ps = pyb.tile([P, HN], fp32, tag="ybp")
                    nc.tensor.matmul(
                        ps[:, :],
                        coef_sb[0:2, C_LY + f * P : C_LY + (f + 1) * P],
                        coef_sb[0:2, C_RY + hf * HN : C_RY + (hf + 1) * HN],
                        start=True, stop=True,
                    )
                    nc.scalar.copy(
                        yb[:, f * W + hf * HN : f * W + (hf + 1) * HN], ps[:, :]
                    )

            # ---- main loop: one fused op per [128, 1024] output tile.
            # f0 uses 1 MiB groups on sync so the output stream starts
            # early; the rest are 2 MiB groups byte-balanced over the
            # three DMA issue paths (sync ~12.2, gpsimd ~12, scalar
            # x + ~8 MiB).  The scalar-ring dispatches sit after all ACT
            # PSUM->SBUF copies in ACT program order. ----
            ring = {
                (1, 0): nc.gpsimd, (2, 0): nc.gpsimd, (3, 0): nc.gpsimd,
                (4, 0): nc.gpsimd, (5, 0): nc.gpsimd, (6, 0): nc.gpsimd,
                (1, 1): nc.sync, (3, 1): nc.sync, (5, 1): nc.sync,
                (7, 1): nc.sync,
                (2, 1): nc.scalar, (4, 1): nc.scalar, (6, 1): nc.scalar,
                (7, 0): nc.scalar,
            }
            group_plan = [(0, h, 2, nc.sync) for h in range(4)]
            for f in range(1, F_LOC):
                for h in range(NB // GH):
                    group_plan.append((f, h, GH, ring[(f, h)]))
            out_r = out_d.rearrange("f (g p) j -> f p g j", p=P)
            for f, h, gh, dma_eng in group_plan:
                big = opool.tile([P, GH * W], fp32, tag="big")
                for k in range(gh):
                    b = h * gh + k
                    q = f * NB + b
                    nc.vector.scalar_tensor_tensor(
                        big[:, k * W : (k + 1) * W],
                        x_sb[:, b * W : (b + 1) * W],
                        xc[:, q : q + 1],
                        yb[:, f * W : (f + 1) * W],
                        AOP.add,
                        AOP.add,
                    )
                dma_eng.dma_start(
                    out=out_r[f, :, h * gh : (h + 1) * gh, :],
                    in_=big[:, : gh * W].rearrange("p (g j) -> p g j", j=W),
                )
    nc.compile()
    return nc


def _get_module():
    if "nc" not in _module_cache:
        _module_cache["nc"] = _build_module()
    return _module_cache["nc"]


def _host_inputs(x, filters):
    x = np.asarray(x, dtype=np.float32)
    filters = np.asarray(filters, dtype=np.float32).reshape(F_TOTAL)
    # pre-transpose x to the SBUF layout [128, 8*1024] (block b at cols b*W)
    xr = np.ascontiguousarray(
        x.reshape(NB, P, W).transpose(1, 0, 2).reshape(P, NB * W)
    )
    c = np.cos(filters)
    s = np.sin(filters)
    half = np.float32(0.5)
    denom = np.float32(W) * np.sqrt(np.float32(2.0))
    idx = (np.arange(W, dtype=np.float32) - np.float32(W / 2 - 0.5)) / denom
    idxcol = idx.reshape(NB, P).T  # [128, 8]
    in_maps = []
    for core in range(N_CORES):
        sl = slice(core * F_LOC, (core + 1) * F_LOC)
        cl, sll = c[sl], s[sl]
        coef = np.zeros((9, COEFW), dtype=np.float32)
        # Y: lhsT rows (-0.5 s_f, 0.5 s_f^2) replicated over p
        coef[0, C_LY : C_LY + F_LOC * P] = np.repeat(-half * sll, P)
        coef[1, C_LY : C_LY + F_LOC * P] = np.repeat(half * sll * sll, P)
        # Y basis rows [idx; ones]
        coef[0, C_RY : C_RY + W] = idx
        coef[1, C_RY : C_RY + W] = 1.0
        # X basis [ones; idx column blocks]
        coef[0, C_LX : C_LX + P] = 1.0
        for b in range(NB):
            coef[1 + b, C_LX : C_LX + P] = idxcol[:, b]
        # X coeffs: col f*NB+b -> (0.5 c_f^2) + idxcol_b * (-0.5 c_f)
        coef[0, C_RX : C_RX + F_LOC * NB] = np.repeat(half * cl * cl, NB)
        for b in range(NB):
            coef[1 + b, C_RX + b : C_RX + F_LOC * NB : NB] = -half * cl
        in_maps.append({"x": xr, "coef": coef})
    return in_maps


def kernel(x, filters):
    global LAST_RESULT
    import concourse.bass_utils as bass_utils

    nc = _get_module()
    in_maps = _host_inputs(x, filters)
    res = bass_utils.run_bass_kernel_spmd(
        nc,
        in_maps,
        core_ids=list(range(N_CORES)),
        trace=TRACE,
        stitch_traces=False,
    )
    LAST_RESULT = res
    return np.concatenate([r["out"] for r in res.results], axis=0)


# revision 26
# speedup vs baseline: 1.0060x; 1.0060x over previous
"""Trainium2 Bass kernel for nn_DirectionalDiagram.

out[f, i, j] = X[f, i] + Y[f, j] + x[i, j]        f in [64], i,j in [1024]
  X[f, i] = (cos(t_f) - idx[i]) * 0.5 * cos(t_f) = 0.5c^2 - 0.5c*idx[i]
  Y[f, j] = (sin(t_f) - idx[j]) * 0.5 * sin(t_f) = 0.5s^2 - 0.5s*idx[j]
  idx[i]  = (i - 511.5) / (1024 * sqrt(2))

Sharding: the filter axis is split across the 8 NeuronCores (8 filters per
core); x is replicated.  Per core the kernel is output-bandwidth bound
(32 MiB of f32 writes); the whole computation is a single fused DVE
scalar_tensor_tensor per [128, 1024] output tile:
    out_tile = (x_tile + X_col[f,b]) + YB[f]
where X_col is a per-partition scalar column and YB[f] is Y[f, :]
broadcast across partitions.  X_col and YB are produced by TensorE
matmuls from host-prepared affine coefficients (idx columns x per-filter
(-0.5s, 0.5s^2) etc.) into PSUM + ScalarE copies to SBUF, so the DVE
(and its SBUF ports) runs nothing but the 64 fused main-loop ops.
Output DMAs are byte-balanced over three DMA issue paths (sync HWDGE,
scalar HWDGE, gpsimd SWDGE) that share the 16 SDMA engines.
"""

import numpy as np

W = 1024          # image side
P = 128           # SBUF partitions
NB = W // P       # 8 row-blocks
F_TOTAL = 64
N_CORES = 8
F_LOC = F_TOTAL // N_CORES   # 8 filters per core
GH = 4            # row-blocks per output DMA (2 MiB per dma_start)
HN = 512          # matmul free-dim chunk (one PSUM bank)
# coef input [9, 2240]: per-filter Y coeffs | Y basis | X basis | X coeffs
C_LY = 0          # rows 0-1, cols [0:1024):     lhsT_Y[., f*128+p]
C_RY = 1024       # rows 0-1, cols [1024:2048):  rhs_Y = [idx; ones]
C_LX = 2048       # rows 0-8, cols [2048:2176):  lhsT_X = [ones; idx cols]
C_RX = 2176       # rows 0-8, cols [2176:2240):  rhs_X coeffs
COEFW = C_RX + F_LOC * NB

TRACE = False     # set by test harness to capture an NTFF profile
LAST_RESULT = None

_module_cache = {}


def _build_module():
    import concourse.bacc as bacc
    import concourse.mybir as mybir
    from concourse import tile

    fp32 = mybir.dt.float32
    AOP = mybir.AluOpType

    nc = bacc.Bacc("TRN2", target_bir_lowering=False, debug=False)
    x_d = nc.dram_tensor("x", [P, NB * W], fp32, kind="ExternalInput").ap()
    coef_d = nc.dram_tensor("coef", [9, COEFW], fp32, kind="ExternalInput").ap()
    out_d = nc.dram_tensor("out", [F_LOC, W, W], fp32, kind="ExternalOutput").ap()

    with tile.TileContext(nc) as tc:
        with (
            tc.tile_pool(name="const", bufs=1) as cpool,
            tc.tile_pool(name="outp", bufs=7) as opool,
            tc.tile_pool(name="pxc", bufs=1, space="PSUM") as pxc,
            tc.tile_pool(name="pyb", bufs=4, space="PSUM") as pyb,
        ):
            # ---- coef first on the sync ring: it gates everything ----
            coef_sb = cpool.tile([9, COEFW], fp32)
            nc.sync.dma_start(out=coef_sb[:, :], in_=coef_d[:, :])

            # ---- x (host-pretransposed to [128, 8*1024]) in quarters on
            # the scalar ring ----
            x_sb = cpool.tile([P, NB * W], fp32)
            QW = NB * W // 4
            for q in range(4):
                nc.scalar.dma_start(
                    out=x_sb[:, q * QW : (q + 1) * QW],
                    in_=x_d[:, q * QW : (q + 1) * QW],
                )

            # ---- X columns xc[p, f*NB+b] via one K=9 matmul:
            # [ones; idxcol_b'] rows x host coeffs (0.5c^2, -0.5c) ----
            psx = pxc.tile([P, F_LOC * NB], fp32)
            nc.tensor.matmul(
                psx[:, :],
                coef_sb[:, C_LX : C_LX + P],
                coef_sb[:, C_RX : C_RX + F_LOC * NB],
                start=True, stop=True,
            )
            xc = cpool.tile([P, F_LOC * NB], fp32)
            nc.scalar.copy(xc[:, :], psx[:, :])

            # ---- YB[f] = Y[f, :] broadcast to 128 partitions via K=2
            # matmuls: [-0.5s_f; 0.5s_f^2] rows x [idx; ones] ----
            yb = cpool.tile([P, F_LOC * W], fp32)
            for f in range(F_LOC):
                for hf in range(W // HN):
                    ps = pyb.tile([P, HN], fp32, tag="ybp")
                    nc.tensor.matmul(
                        ps[:, :],
                        coef_sb[0:2, C_LY + f * P : C_LY + (f + 1) * P],
                        coef_sb[0:2, C_RY + hf * HN : C_RY + (hf + 1) * HN],
                        start=True, stop=True,
                    )
                    nc.scalar.copy(
                        yb[:, f * W + hf * HN : f * W + (hf + 1) * HN], ps[:, :]
                    )

            # ---- main loop: one fused op per [128, 1024] output tile.
            # f0 uses 1 MiB groups on sync so the output stream starts
            # early; the rest are 2 MiB groups byte-balanced over the
            # three DMA issue paths (sync ~12.2, gpsimd ~12, scalar
            # x + ~8 MiB).  The scalar-ring dispatches sit after all ACT
            # PSUM->SBUF copies in ACT program order. ----
            ring = {
                (1, 0): nc.gpsimd, (2, 0): nc.gpsimd, (3, 0): nc.gpsimd,
                (4, 0): nc.gpsimd, (5, 0): nc.gpsimd,
                (1, 1): nc.sync, (3, 1): nc.sync, (5, 1): nc.sync,
                (2, 1): nc.scalar, (4, 1): nc.scalar, (6, 1): nc.scalar,
                (6, 0): nc.scalar,
            }
            group_plan = [(0, h, 2, nc.sync) for h in range(4)]
            for f in range(1, F_LOC - 1):
                for h in range(NB // GH):
                    group_plan.append((f, h, GH, ring[(f, h)]))
            # last filter: 1 MiB groups fanned over all three paths so the
            # final flush drains in parallel
            group_plan += [
                (7, 0, 2, nc.gpsimd),
                (7, 1, 2, nc.sync),
                (7, 2, 2, nc.scalar),
                (7, 3, 2, nc.sync),
            ]
            out_r = out_d.rearrange("f (g p) j -> f p g j", p=P)
            for f, h, gh, dma_eng in group_plan:
                big = opool.tile([P, GH * W], fp32, tag="big")
                for k in range(gh):
                    b = h * gh + k
                    q = f * NB + b
                    nc.vector.scalar_tensor_tensor(
                        big[:, k * W : (k + 1) * W],
                        x_sb[:, b * W : (b + 1) * W],
                        xc[:, q : q + 1],
                        yb[:, f * W : (f + 1) * W],
                        AOP.add,
                        AOP.add,
                    )
                dma_eng.dma_start(
                    out=out_r[f, :, h * gh : (h + 1) * gh, :],
                    in_=big[:, : gh * W].rearrange("p (g j) -> p g j", j=W),
                )
    nc.compile()
    return nc


def _get_module():
    if "nc" not in _module_cache:
        _module_cache["nc"] = _build_module()
    return _module_cache["nc"]


def _host_inputs(x, filters):
    x = np.asarray(x, dtype=np.float32)
    filters = np.asarray(filters, dtype=np.float32).reshape(F_TOTAL)
    # pre-transpose x to the SBUF layout [128, 8*1024] (block b at cols b*W)
    xr = np.ascontiguousarray(
        x.reshape(NB, P, W).transpose(1, 0, 2).reshape(P, NB * W)
    )
    c = np.cos(filters)
    s = np.sin(filters)
    half = np.float32(0.5)
    denom = np.float32(W) * np.sqrt(np.float32(2.0))
    idx = (np.arange(W, dtype=np.float32) - np.float32(W / 2 - 0.5)) / denom
    idxcol = idx.reshape(NB, P).T  # [128, 8]
    in_maps = []
    for core in range(N_CORES):
        sl = slice(core * F_LOC, (core + 1) * F_LOC)
        cl, sll = c[sl], s[sl]
        coef = np.zeros((9, COEFW), dtype=np.float32)
        # Y: lhsT rows (-0.5 s_f, 0.5 s_f^2) replicated over p
        coef[0, C_LY : C_LY + F_LOC * P] = np.repeat(-half * sll, P)
        coef[1, C_LY : C_LY + F_LOC * P] = np.repeat(half * sll * sll, P)
        # Y basis rows [idx; ones]
        coef[0, C_RY : C_RY + W] = idx
        coef[1, C_RY : C_RY + W] = 1.0
        # X basis [ones; idx column blocks]
        coef[0, C_LX : C_LX + P] = 1.0
        for b in range(NB):
            coef[1 + b, C_LX : C_LX + P] = idxcol[:, b]
        # X coeffs: col f*NB+b -> (0.5 c_f^2) + idxcol_b * (-0.5 c_f)
        coef[0, C_RX : C_RX + F_LOC * NB] = np.repeat(half * cl * cl, NB)
        for b in range(NB):
            coef[1 + b, C_RX + b : C_RX + F_LOC * NB : NB] = -half * cl
        in_maps.append({"x": xr, "coef": coef})
    return in_maps


def kernel(x, filters):
    global LAST_RESULT
    import concourse.bass_utils as bass_utils

    nc = _get_module()
    in_maps = _host_inputs(x, filters)
    res = bass_utils.run_bass_kernel_spmd(
        nc,
        in_maps,
        core_ids=list(range(N_CORES)),
        trace=TRACE,
        stitch_traces=False,
    )
    LAST_RESULT = res
    return np.concatenate([r["out"] for r in res.results], axis=0)
